# revision 19
# baseline (speedup 1.0000x reference)
"""Trainium2 Bass kernel for the GroupNorm + single-head spatial attention block.

Reference computation (per batch b):
    n  = GroupNorm(x, groups=4) * gn_w + gn_b          x: [C=256, N=1024]
    Q  = Wq @ n + bq ; K = Wk @ n + bk ; V = Wv @ n + bv
    S  = Q^T K / sqrt(C)                                [N, N]
    A  = softmax(S, axis=-1)
    U  = V @ A^T                                        [C, N]
    y  = x + Wo @ U + bo

Strategy (data-parallel over batch, 2 batches per NeuronCore, 8 cores):
  - matmul operands in bf16 (1 cycle/row on the PE, fp32 PSUM accumulation);
    everything else (stats, softmax denominator, residual) in fp32.
  - S is computed TRANSPOSED (S^T = K_tile^T @ Q, j on partitions) so the
    exp result E^T = exp(S^T/16) feeds U = V @ E^T directly (contraction
    over j on partitions).  No [N,N] transpose anywhere.
  - softmax skips the max-subtraction (|S|/16 is O(1), exp is safe).  The
    denominator (sum over j = partitions) is accumulated on DVE across
    j-tiles, then ONE ones[128,128] matmul both reduces over partitions and
    broadcasts to all 128 partitions.  The reciprocal is applied AFTER the
    Wo projection (per-output-column scaling commutes through the V
    contraction and Wo), so the PE never waits on it.
  - GroupNorm stats: bn_stats/bn_aggr per partition, then a [128,2]
    indicator matmul reduces (mean, E[x^2]) over each group's 64 partitions
    and a [2,128] indicator matmul broadcasts (mean, rstd) back.
  - engine balance: PE ~44us of matmul; ACT: exp, V^T copies, x+bo;
    DVE: bn stats, Q/K bias copies, colsum accumulation, reciprocal,
    U*recip; GpSimd: groupnorm apply, final residual add.
  - DMA: x on the sync (HWDGE) queue first; weights in parallel on the
    gpsimd (SWDGE) queue so compute starts ~3us in.
"""

import os
import numpy as np

import concourse.bass as bass
import concourse.bacc as bacc
import concourse.tile as tile
import concourse.bass_utils as bass_utils
from concourse import mybir
from concourse.alu_op_type import AluOpType

P = 128
B, C, H, W = 16, 256, 32, 32
N = H * W                 # 1024
N_CORES = 8
BPC = B // N_CORES        # batches per core
CT = C // P               # 2 c-tiles
JT = N // P               # 8 j-tiles
FH = 512                  # free-dim half (one PSUM bank of fp32)
IH = N // FH              # 2 i-halves
GROUPS = 4
GSIZE = C // GROUPS       # 64 channels per group
EPS = 1e-5
SCALE = 1.0 / float(np.sqrt(C))

F32 = mybir.dt.float32
BF16 = mybir.dt.bfloat16

AF = mybir.ActivationFunctionType

# matmul operand dtype: bf16 (fast) or f32 (exact, 4 cycles/row)
MODE = os.environ.get("ATTN_DT", "bf16")
MM_DT = {"bf16": BF16, "f32": F32}[MODE]


def _np_mm_dt():
    import ml_dtypes
    return {"bf16": ml_dtypes.bfloat16, "f32": np.float32}[MODE]


def _build_gn(nc, tc, pools, aps, b):
    """GroupNorm + xb for batch b (hoisted so both batches' DVE work runs
    up front, keeping the later batch's matmuls ready when the PE frees)."""
    (consts, xpool, npool, qkpool, vtpool, etpool, accpool, rcpool, upool,
     ypool, xbpool, small, p_st, p_u, p_misc) = pools

    sl = [slice(ih * FH, (ih + 1) * FH) for ih in range(IH)]
    x_t = aps["x_sb"][b]          # list of CT tiles [P, N]

    # ---- GroupNorm (both c-tiles' chains batched into strided ops) ----
    n_sb = npool.tile([P, CT, N], MM_DT, tag="n")
    mv_all = small.tile([P, CT, 2], F32, tag="mv")
    for t in range(CT):
        resh = x_t[t][:].rearrange("p (s f) -> p s f", f=FH)
        stats6 = small.tile([P, IH, 6], F32, tag=f"stats6_{t}")
        for s in range(IH):
            nc.vector.bn_stats(out=stats6[:, s, :], in_=resh[:, s, :])
        nc.vector.bn_aggr(out=mv_all[:, t, :], in_=stats6[:])
    # pq = (mean, E[x^2]) per partition, both tiles side by side
    pq = small.tile([P, CT, 2], F32, tag="pq")
    nc.vector.tensor_copy(pq[:, :, 0], mv_all[:, :, 0])
    nc.vector.tensor_mul(pq[:, :, 1], mv_all[:, :, 0], mv_all[:, :, 0])
    nc.vector.tensor_add(pq[:, :, 1], pq[:, :, 1], mv_all[:, :, 1])
    # group-reduce over partitions: [2, (t,k)] = (mean_g, E[x^2]_g)
    stats_ps = p_misc.tile([2, CT, 2], F32, tag="m")
    nc.tensor.matmul(stats_ps[:], aps["ind_fwd"][:], pq[:],
                     start=True, stop=True)
    s_sb = small.tile([2, CT, 2], F32, tag="s2")
    nc.vector.tensor_copy(s_sb[:], stats_ps[:])
    msq = small.tile([2, CT], F32, tag="msq")
    nc.vector.tensor_mul(msq[:], s_sb[:, :, 0], s_sb[:, :, 0])
    nc.vector.tensor_sub(msq[:], s_sb[:, :, 1], msq[:])         # var
    nc.scalar.activation(out=msq[:], in_=msq[:], func=AF.Sqrt,
                         bias=aps["eps"][:])                    # sqrt(var+eps)
    nc.vector.reciprocal(out=s_sb[:, :, 1], in_=msq[:])         # rstd
    # broadcast (mean, rstd) to the 128 partitions
    bc_ps = p_misc.tile([P, CT, 2], F32, tag="m")
    nc.tensor.matmul(bc_ps[:], aps["ind_bwd"][:], s_sb[:],
                     start=True, stop=True)
    # fold gamma/beta: n = x * (rstd*w) + (b - mean*rstd*w)
    sc = small.tile([P, CT, 2], F32, tag="sc")
    nc.vector.tensor_mul(sc[:, :, 0], bc_ps[:, :, 1], aps["gnw"])
    nc.vector.tensor_mul(sc[:, :, 1], bc_ps[:, :, 0], sc[:, :, 0])
    nc.vector.tensor_sub(sc[:, :, 1], aps["gnb"], sc[:, :, 1])
    for t in range(CT):
        nc.scalar.activation(out=n_sb[:, t, :], in_=x_t[t][:],
                             func=AF.Identity,
                             scale=sc[:, t, 0:1], bias=sc[:, t, 1:2])
    aps.setdefault("n_sb", {})[b] = n_sb


def _build_attn(nc, tc, pools, aps, b):
    """Projections + attention + output for batch b."""
    (consts, xpool, npool, qkpool, vtpool, etpool, accpool, rcpool, upool,
     ypool, xbpool, small, p_st, p_u, p_misc) = pools

    sl = [slice(ih * FH, (ih + 1) * FH) for ih in range(IH)]
    x_t = aps["x_sb"][b]
    n_sb = aps["n_sb"][b]

    # ---- merged QK projection: S^T = n^T (Wk^T Wq) n, so compute
    # P1 = M n + v with M = Wk^T Wq and v = Wk^T bq (both host-side).
    # The bk/bq cross terms are constant per softmax row and cancel.
    p1_sb = qkpool.tile([P, CT, N], MM_DT, tag="p1")
    for ot in range(CT):
        for ih in range(IH):
            pr_ps = p_misc.tile([P, FH], F32, tag="m")
            for kt in range(CT):
                nc.tensor.matmul(
                    pr_ps[:],
                    aps["wm"][:, kt, ot * P:(ot + 1) * P],
                    n_sb[:, kt, sl[ih]],
                    start=(kt == 0), stop=(kt == CT - 1))
            nc.scalar.activation(
                out=p1_sb[:, ot, sl[ih]], in_=pr_ps[:],
                func=AF.Identity, bias=aps["vq"][:, ot:ot + 1])

    # ---- V^T: [N, C] (j on partitions), computed directly as n^T @ Wv^T ----
    # (bias bv is folded into the residual on the host: softmax rows sum to 1,
    #  so V*A^T with V = V0 + bv x 1 contributes exactly Wo@bv per channel.)
    vt_sb = vtpool.tile([P, JT, C], MM_DT, tag="vt")
    for jt in range(JT):
        vt_ps = p_misc.tile([P, C], F32, tag="m")
        for kt in range(CT):
            nc.tensor.matmul(vt_ps[:],
                             n_sb[:, kt, jt * P:(jt + 1) * P],
                             aps["wv"][:, kt, :],
                             start=(kt == 0), stop=(kt == CT - 1))
        nc.vector.tensor_copy(vt_sb[:, jt, :], vt_ps[:])

    # ---- attention: S^T -> exp -> (colsum, U-accumulate) per j-tile ----
    u_ps = [p_u.tile([P, FH], F32, tag="u", name=f"u_ps{b}_{i}")
            for i in range(CT * IH)]
    acc_a = accpool.tile([P, N], MM_DT, tag="acc_a")
    acc_b = accpool.tile([P, N], MM_DT, tag="acc_b")
    for jt in range(JT):
        et = etpool.tile([P, N], MM_DT, tag="et")
        for ih in range(IH):
            st_ps = p_st.tile([P, FH], F32, tag="st")
            for kt in range(CT):
                nc.tensor.matmul(
                    st_ps[:],
                    n_sb[:, kt, jt * P:(jt + 1) * P],
                    p1_sb[:, kt, sl[ih]],
                    start=(kt == 0), stop=(kt == CT - 1))
            nc.scalar.activation(out=et[:, sl[ih]], in_=st_ps[:],
                                 func=AF.Exp, scale=SCALE)
        if jt == 0:
            nc.vector.tensor_copy(acc_a[:], et[:])
        elif jt == 1:
            nc.vector.tensor_copy(acc_b[:], et[:])
        elif jt % 2 == 0:
            nc.vector.tensor_add(acc_a[:], acc_a[:], et[:])
        else:
            nc.vector.tensor_add(acc_b[:], acc_b[:], et[:])
        for ci in range(CT):
            for ih in range(IH):
                nc.tensor.matmul(
                    u_ps[ci * IH + ih][:],
                    vt_sb[:, jt, ci * P:(ci + 1) * P],
                    et[:, sl[ih]],
                    start=(jt == 0), stop=(jt == JT - 1))

    # ---- xb = x + bo (per-partition bias), used by the final residual ----
    xb_sb = xbpool.tile([P, CT, N], F32, tag="xb")
    for ot in range(CT):
        nc.scalar.activation(out=xb_sb[:, ot, :], in_=x_t[ot][:],
                             func=AF.Identity, bias=aps["bo"][:, ot:ot + 1])

    # ---- denominator: ones[128,128] matmul = partition-reduce + broadcast
    rc_sb = rcpool.tile([P, N], F32, tag="rc")
    rscr = rcpool.tile([P, FH], F32, tag="rscr")
    for ih in range(IH):
        cs_ps = p_misc.tile([P, FH], F32, tag="m")
        nc.tensor.matmul(cs_ps[:], aps["ones_sq"][:], acc_a[:, sl[ih]],
                         start=True, stop=False)
        nc.tensor.matmul(cs_ps[:], aps["ones_sq"][:], acc_b[:, sl[ih]],
                         start=False, stop=True)
        nc.vector.reciprocal_approx_accurate(out=rc_sb[:, sl[ih]],
                                             in_=cs_ps[:], scratch=rscr[:])

    # ---- copy (unnormalized) U to SBUF; normalization is deferred past Wo
    u_sb = upool.tile([P, CT, N], MM_DT, tag="u_sb")
    for ci in range(CT):
        for ih in range(IH):
            nc.scalar.activation(out=u_sb[:, ci, sl[ih]],
                                 in_=u_ps[ci * IH + ih][:], func=AF.Copy)

    # ---- output projection; then y = (Wo U') + (x + bo) ----
    y_sb = ypool.tile([P, CT, N], F32, tag="y")
    for ot in range(CT):
        for ih in range(IH):
            o_ps = p_misc.tile([P, FH], F32, tag="m")
            for ci in range(CT):
                nc.tensor.matmul(
                    o_ps[:],
                    aps["wo"][:, ci, ot * P:(ot + 1) * P],
                    u_sb[:, ci, sl[ih]],
                    start=(ci == 0), stop=(ci == CT - 1))
            nc.vector.tensor_mul(y_sb[:, ot, sl[ih]], o_ps[:],
                                 rc_sb[:, sl[ih]])
            nc.gpsimd.tensor_add(y_sb[:, ot, sl[ih]], y_sb[:, ot, sl[ih]],
                                 xb_sb[:, ot, sl[ih]])
            dma_eng = nc.sync if (ot + ih) % 2 == 0 else nc.scalar
            dma_eng.dma_start(out=aps["y"][b][:, ot, sl[ih]],
                              in_=y_sb[:, ot, sl[ih]])


def _build():
    nc = bacc.Bacc("TRN2", target_bir_lowering=False, debug=False,
                   enable_asserts=False, num_devices=N_CORES)

    x_d = nc.dram_tensor("x", [BPC, C, N], F32, kind="ExternalInput")
    y_d = nc.dram_tensor("y", [BPC, C, N], F32, kind="ExternalOutput")
    wall_d = nc.dram_tensor("wall", [3, C, C], MM_DT, kind="ExternalInput")
    cpack_d = nc.dram_tensor("cpack", [P, 16], F32, kind="ExternalInput")

    with tile.TileContext(nc) as tc:
        with (
            tc.tile_pool(name="consts", bufs=1) as consts,
            tc.tile_pool(name="xpool", bufs=2) as xpool,
            tc.tile_pool(name="npool", bufs=2) as npool,
            tc.tile_pool(name="qkpool", bufs=2) as qkpool,
            tc.tile_pool(name="vtpool", bufs=2) as vtpool,
            tc.tile_pool(name="etpool", bufs=3) as etpool,
            tc.tile_pool(name="accpool", bufs=2) as accpool,
            tc.tile_pool(name="rcpool", bufs=2) as rcpool,
            tc.tile_pool(name="upool", bufs=2) as upool,
            tc.tile_pool(name="ypool", bufs=2) as ypool,
            tc.tile_pool(name="xbpool", bufs=2) as xbpool,
            tc.tile_pool(name="small", bufs=4) as small,
            tc.tile_pool(name="p_st", bufs=2, space="PSUM") as p_st,
            tc.tile_pool(name="p_u", bufs=CT * IH, space="PSUM") as p_u,
            tc.tile_pool(name="p_misc", bufs=2, space="PSUM") as p_misc,
        ):
            aps = {}
            aps["x"] = x_d.ap().rearrange("b (t p) n -> b p t n", p=P)
            aps["y"] = y_d.ap().rearrange("b (t p) n -> b p t n", p=P)

            # x first (gates groupnorm) on the HWDGE sync queue
            # one packed const DMA: [P,16] f32 holds gnw|gnb|vq|bo|ind_fwd
            # (cols 0..11) and ind_bwd packed transposed in cols 12..13.
            # These are tiny and gate the groupnorm tail: issue them FIRST.
            cp = consts.tile([P, 16], F32, tag="cpack")
            nc.sync.dma_start(out=cp[:], in_=cpack_d.ap())
            aps["gnw"] = cp[:, 0:2]
            aps["gnb"] = cp[:, 2:4]
            aps["vq"] = cp[:, 4:6]
            aps["bo"] = cp[:, 8:10]
            aps["ind_fwd"] = cp[:, 10:12]
            ind_bwd = consts.tile([2, P], F32, tag="ind_bwd")
            nc.sync.dma_start(
                out=ind_bwd[:],
                in_=bass.AP(tensor=cpack_d, offset=12, ap=[[1, 2], [16, P]]))
            aps["ind_bwd"] = ind_bwd

            aps["x_sb"] = []
            for b in range(BPC):
                tiles = []
                for t in range(CT):
                    x_tt = xpool.tile([P, N], F32, tag=f"x{t}",
                                      name=f"x_sb{b}_{t}")
                    dma_eng = nc.sync if t == 0 else nc.scalar
                    dma_eng.dma_start(out=x_tt[:], in_=aps["x"][b][:, t, :])
                    tiles.append(x_tt)
                aps["x_sb"].append(tiles)
            ones_sq = consts.tile([P, P], MM_DT, tag="ones_sq")
            nc.gpsimd.memset(ones_sq[:], 1.0)
            aps["ones_sq"] = ones_sq
            eps_t = consts.tile([2, 1], F32, tag="eps")
            nc.vector.memset(eps_t[:], EPS)
            aps["eps"] = eps_t
            warm = consts.tile([2, 4], F32, tag="actwarm")
            for wi, fn in enumerate((AF.Sqrt, AF.Identity, AF.Exp, AF.Copy)):
                nc.scalar.activation(out=warm[:, wi:wi + 1],
                                     in_=eps_t[:], func=fn)

            # all weights in ONE DMA on the scalar HWDGE ring
            wall_t = consts.tile([P, 3, CT, C], MM_DT, tag="wall")
            nc.scalar.dma_start(
                out=wall_t[:],
                in_=wall_d.ap().rearrange("w (t p) o -> p w t o", p=P))
            for wi, dst in enumerate(("wm", "wv", "wo")):
                aps[dst] = wall_t[:, wi]

            pools = (consts, xpool, npool, qkpool, vtpool, etpool, accpool,
                     rcpool, upool, ypool, xbpool, small, p_st, p_u, p_misc)
            for b in range(BPC):
                _build_gn(nc, tc, pools, aps, b)
            for b in range(BPC):
                _build_attn(nc, tc, pools, aps, b)

    nc.compile()
    return nc


_NC = None


def _get_nc():
    global _NC
    if _NC is None:
        _NC = _build()
    return _NC


def _make_in_maps(inputs):
    f32 = lambda a: np.ascontiguousarray(np.asarray(a, dtype=np.float32))
    mmdt = _np_mm_dt()
    wt = lambda a: np.asarray(a, dtype=np.float32).T.astype(mmdt)
    x = f32(inputs["x"]).reshape(B, C, N)
    wq64 = np.asarray(inputs["Wq"], np.float64)
    wk64 = np.asarray(inputs["Wk"], np.float64)
    # M^T = (Wk^T Wq)^T = Wq^T Wk, laid out [c', o] for the lhsT slot
    mT = (wq64.T @ wk64).astype(np.float32).astype(mmdt)
    wall = np.ascontiguousarray(np.stack(
        [np.ascontiguousarray(mT), wt(inputs["Wv"]), wt(inputs["Wo"])]))
    # softmax rows sum to 1 => the bv term reaches y as the constant
    # per-channel vector Wo @ bv; fold it into bo on the host.
    bo_eff = (np.asarray(inputs["bo"], np.float64)
              + np.asarray(inputs["Wo"], np.float64)
              @ np.asarray(inputs["bv"], np.float64)).astype(np.float32)
    pt = lambda a: f32(a).reshape(CT, P).T          # [256] -> [P, CT]
    cpack = np.zeros((P, 16), np.float32)
    cpack[:, 0:2] = pt(inputs["gn_w"])
    cpack[:, 2:4] = pt(inputs["gn_b"])
    vq = wk64.T @ np.asarray(inputs["bq"], np.float64)   # folds bq into P1
    cpack[:, 4:6] = pt(vq.astype(np.float32))
    cpack[:, 8:10] = pt(bo_eff)
    cpack[:GSIZE, 10] = 1.0 / GSIZE                 # ind_fwd
    cpack[GSIZE:, 11] = 1.0 / GSIZE
    cpack[:GSIZE, 12] = 1.0                         # ind_bwd (transposed)
    cpack[GSIZE:, 13] = 1.0
    shared = {"wall": wall, "cpack": cpack}

    in_maps = []
    for m in range(N_CORES):
        im = dict(shared)
        im["x"] = np.ascontiguousarray(x[m * BPC:(m + 1) * BPC])
        in_maps.append(im)
    return in_maps


def _gather(results):
    y = np.concatenate([r["y"] for r in results], axis=0)
    return np.ascontiguousarray(y.reshape(B, C, H, W).astype(np.float32))


def kernel(**inputs):
    nc = _get_nc()
    res = bass_utils.run_bass_kernel_spmd(nc, _make_in_maps(inputs),
                                          core_ids=list(range(N_CORES)))
    return _gather(res.results)


def _ensure_ntff_hook():
    """The agent image lacks antenv.axon_hooks; synthesize it and install the
    ctypes-based NTFF hook from trn_agent_boot so trace=True works locally."""
    import sys
    import types
    try:
        from antenv.axon_hooks import get_axon_ntff_profile_hook  # noqa: F401
        return
    except ImportError:
        pass
    hook = None
    try:
        from trn_agent_boot.trn_boot import _ntff_profile_via_ctypes
        hook = _ntff_profile_via_ctypes("/opt/axon/libaxon_pjrt.so")
    except Exception:
        hook = None
    mod = types.ModuleType("antenv.axon_hooks")
    mod.get_axon_ntff_profile_hook = lambda: hook
    mod.set_axon_ntff_profile_hook = lambda h: None
    sys.modules["antenv.axon_hooks"] = mod
    # keep artifacts local: no bucket in this sandbox
    bass_utils.upload_artifacts = lambda d: d


def kernel_traced(**inputs):
    """Returns (output, exec_time_ns, trace_path) using NTFF profiling."""
    _ensure_ntff_hook()
    nc = _get_nc()
    res = bass_utils.run_bass_kernel_spmd(nc, _make_in_maps(inputs),
                                          core_ids=list(range(N_CORES)),
                                          trace=True)
    trace_path = None
    if res.instructions_and_trace is not None:
        trace_path = res.instructions_and_trace[1]
    return _gather(res.results), res.exec_time_ns, trace_path


# revision 20
# speedup vs baseline: 1.0230x; 1.0230x over previous
"""Trainium2 Bass kernel for the GroupNorm + single-head spatial attention block.

Reference computation (per batch b):
    n  = GroupNorm(x, groups=4) * gn_w + gn_b          x: [C=256, N=1024]
    Q  = Wq @ n + bq ; K = Wk @ n + bk ; V = Wv @ n + bv
    S  = Q^T K / sqrt(C)                                [N, N]
    A  = softmax(S, axis=-1)
    U  = V @ A^T                                        [C, N]
    y  = x + Wo @ U + bo

Strategy (data-parallel over batch, 2 batches per NeuronCore, 8 cores):
  - matmul operands in bf16 (1 cycle/row on the PE, fp32 PSUM accumulation);
    everything else (stats, softmax denominator, residual) in fp32.
  - S is computed TRANSPOSED (S^T = K_tile^T @ Q, j on partitions) so the
    exp result E^T = exp(S^T/16) feeds U = V @ E^T directly (contraction
    over j on partitions).  No [N,N] transpose anywhere.
  - softmax skips the max-subtraction (|S|/16 is O(1), exp is safe).  The
    denominator (sum over j = partitions) is accumulated on DVE across
    j-tiles, then ONE ones[128,128] matmul both reduces over partitions and
    broadcasts to all 128 partitions.  The reciprocal is applied AFTER the
    Wo projection (per-output-column scaling commutes through the V
    contraction and Wo), so the PE never waits on it.
  - GroupNorm stats: bn_stats/bn_aggr per partition, then a [128,2]
    indicator matmul reduces (mean, E[x^2]) over each group's 64 partitions
    and a [2,128] indicator matmul broadcasts (mean, rstd) back.
  - engine balance: PE ~44us of matmul; ACT: exp, V^T copies, x+bo;
    DVE: bn stats, Q/K bias copies, colsum accumulation, reciprocal,
    U*recip; GpSimd: groupnorm apply, final residual add.
  - DMA: x on the sync (HWDGE) queue first; weights in parallel on the
    gpsimd (SWDGE) queue so compute starts ~3us in.
"""

import os
import numpy as np

import concourse.bass as bass
import concourse.bacc as bacc
import concourse.tile as tile
import concourse.bass_utils as bass_utils
from concourse import mybir
from concourse.alu_op_type import AluOpType

P = 128
B, C, H, W = 16, 256, 32, 32
N = H * W                 # 1024
N_CORES = 8
BPC = B // N_CORES        # batches per core
CT = C // P               # 2 c-tiles
JT = N // P               # 8 j-tiles
FH = 512                  # free-dim half (one PSUM bank of fp32)
IH = N // FH              # 2 i-halves
GROUPS = 4
GSIZE = C // GROUPS       # 64 channels per group
EPS = 1e-5
SCALE = 1.0 / float(np.sqrt(C))

F32 = mybir.dt.float32
BF16 = mybir.dt.bfloat16

AF = mybir.ActivationFunctionType

# matmul operand dtype: bf16 (fast) or f32 (exact, 4 cycles/row)
MODE = os.environ.get("ATTN_DT", "bf16")
MM_DT = {"bf16": BF16, "f32": F32}[MODE]


def _np_mm_dt():
    import ml_dtypes
    return {"bf16": ml_dtypes.bfloat16, "f32": np.float32}[MODE]


def _build_gn(nc, tc, pools, aps, b):
    """GroupNorm + xb for batch b (hoisted so both batches' DVE work runs
    up front, keeping the later batch's matmuls ready when the PE frees)."""
    (consts, xpool, npool, qkpool, vtpool, etpool, accpool, rcpool, upool,
     ypool, xbpool, small, p_st, p_u, p_misc) = pools

    sl = [slice(ih * FH, (ih + 1) * FH) for ih in range(IH)]
    x_t = aps["x_sb"][b]          # list of CT tiles [P, N]

    # ---- GroupNorm (both c-tiles' chains batched into strided ops) ----
    n_sb = npool.tile([P, CT, N], MM_DT, tag="n")
    mv_all = small.tile([P, CT, 2], F32, tag="mv")
    for t in range(CT):
        resh = x_t[t][:].rearrange("p (s f) -> p s f", f=FH)
        stats6 = small.tile([P, IH, 6], F32, tag=f"stats6_{t}")
        for s in range(IH):
            nc.vector.bn_stats(out=stats6[:, s, :], in_=resh[:, s, :])
        nc.vector.bn_aggr(out=mv_all[:, t, :], in_=stats6[:])
    # pq = (mean, E[x^2]) per partition, both tiles side by side
    pq = small.tile([P, CT, 2], F32, tag="pq")
    nc.vector.tensor_copy(pq[:, :, 0], mv_all[:, :, 0])
    nc.vector.tensor_mul(pq[:, :, 1], mv_all[:, :, 0], mv_all[:, :, 0])
    nc.vector.tensor_add(pq[:, :, 1], pq[:, :, 1], mv_all[:, :, 1])
    # group-reduce over partitions: [2, (t,k)] = (mean_g, E[x^2]_g)
    stats_ps = p_misc.tile([2, CT, 2], F32, tag="m")
    nc.tensor.matmul(stats_ps[:], aps["ind_fwd"][:], pq[:],
                     start=True, stop=True)
    s_sb = small.tile([2, CT, 2], F32, tag="s2")
    nc.vector.tensor_copy(s_sb[:], stats_ps[:])
    msq = small.tile([2, CT], F32, tag="msq")
    nc.vector.tensor_mul(msq[:], s_sb[:, :, 0], s_sb[:, :, 0])
    nc.vector.tensor_sub(msq[:], s_sb[:, :, 1], msq[:])         # var
    nc.scalar.activation(out=msq[:], in_=msq[:], func=AF.Sqrt,
                         bias=aps["eps"][:])                    # sqrt(var+eps)
    nc.vector.reciprocal(out=s_sb[:, :, 1], in_=msq[:])         # rstd
    # broadcast (mean, rstd) to the 128 partitions
    bc_ps = p_misc.tile([P, CT, 2], F32, tag="m")
    nc.tensor.matmul(bc_ps[:], aps["ind_bwd"][:], s_sb[:],
                     start=True, stop=True)
    # fold gamma/beta: n = x * (rstd*w) + (b - mean*rstd*w)
    sc = small.tile([P, CT, 2], F32, tag="sc")
    nc.vector.tensor_mul(sc[:, :, 0], bc_ps[:, :, 1], aps["gnw"])
    nc.vector.tensor_mul(sc[:, :, 1], bc_ps[:, :, 0], sc[:, :, 0])
    nc.vector.tensor_sub(sc[:, :, 1], aps["gnb"], sc[:, :, 1])
    for t in range(CT):
        nc.scalar.activation(out=n_sb[:, t, :], in_=x_t[t][:],
                             func=AF.Identity,
                             scale=sc[:, t, 0:1], bias=sc[:, t, 1:2])
    aps.setdefault("n_sb", {})[b] = n_sb


def _build_attn(nc, tc, pools, aps, b):
    """Projections + attention + output for batch b."""
    (consts, xpool, npool, qkpool, vtpool, etpool, accpool, rcpool, upool,
     ypool, xbpool, small, p_st, p_u, p_misc) = pools

    sl = [slice(ih * FH, (ih + 1) * FH) for ih in range(IH)]
    x_t = aps["x_sb"][b]
    n_sb = aps["n_sb"][b]

    # ---- merged QK projection: S^T = n^T (Wk^T Wq) n, so compute
    # P1 = M n + v with M = Wk^T Wq and v = Wk^T bq (both host-side).
    # The bk/bq cross terms are constant per softmax row and cancel.
    p1_sb = qkpool.tile([P, CT, N], MM_DT, tag="p1")
    for ot in range(CT):
        for ih in range(IH):
            pr_ps = p_misc.tile([P, FH], F32, tag="m")
            for kt in range(CT):
                nc.tensor.matmul(
                    pr_ps[:],
                    aps["wm"][:, kt, ot * P:(ot + 1) * P],
                    n_sb[:, kt, sl[ih]],
                    start=(kt == 0), stop=(kt == CT - 1))
            nc.vector.tensor_scalar(
                out=p1_sb[:, ot, sl[ih]], in0=pr_ps[:],
                scalar1=aps["vq"][:, ot:ot + 1], scalar2=None,
                op0=AluOpType.add)

    # ---- V^T: [N, C] (j on partitions), computed directly as n^T @ Wv^T ----
    # (bias bv is folded into the residual on the host: softmax rows sum to 1,
    #  so V*A^T with V = V0 + bv x 1 contributes exactly Wo@bv per channel.)
    vt_sb = vtpool.tile([P, JT, C], MM_DT, tag="vt")
    for jt in range(JT):
        vt_ps = p_misc.tile([P, C], F32, tag="m")
        for kt in range(CT):
            nc.tensor.matmul(vt_ps[:],
                             n_sb[:, kt, jt * P:(jt + 1) * P],
                             aps["wv"][:, kt, :],
                             start=(kt == 0), stop=(kt == CT - 1))
        nc.vector.tensor_copy(vt_sb[:, jt, :], vt_ps[:])

    # ---- attention: S^T -> exp -> (colsum, U-accumulate) per j-tile ----
    u_ps = [p_u.tile([P, FH], F32, tag="u", name=f"u_ps{b}_{i}")
            for i in range(CT * IH)]
    acc_a = accpool.tile([P, N], MM_DT, tag="acc_a")
    acc_b = accpool.tile([P, N], MM_DT, tag="acc_b")
    for jt in range(JT):
        et = etpool.tile([P, N], MM_DT, tag="et")
        for ih in range(IH):
            st_ps = p_st.tile([P, FH], F32, tag="st")
            for kt in range(CT):
                nc.tensor.matmul(
                    st_ps[:],
                    n_sb[:, kt, jt * P:(jt + 1) * P],
                    p1_sb[:, kt, sl[ih]],
                    start=(kt == 0), stop=(kt == CT - 1))
            nc.scalar.activation(out=et[:, sl[ih]], in_=st_ps[:],
                                 func=AF.Exp, scale=SCALE)
        if jt == 0:
            nc.vector.tensor_copy(acc_a[:], et[:])
        elif jt == 1:
            nc.vector.tensor_copy(acc_b[:], et[:])
        elif jt % 2 == 0:
            nc.vector.tensor_add(acc_a[:], acc_a[:], et[:])
        else:
            nc.vector.tensor_add(acc_b[:], acc_b[:], et[:])
        for ci in range(CT):
            for ih in range(IH):
                nc.tensor.matmul(
                    u_ps[ci * IH + ih][:],
                    vt_sb[:, jt, ci * P:(ci + 1) * P],
                    et[:, sl[ih]],
                    start=(jt == 0), stop=(jt == JT - 1))

    # ---- xb = x + bo (per-partition bias), used by the final residual ----
    xb_sb = xbpool.tile([P, CT, N], F32, tag="xb")
    for ot in range(CT):
        nc.vector.tensor_scalar(out=xb_sb[:, ot, :], in0=x_t[ot][:],
                                scalar1=aps["bo"][:, ot:ot + 1], scalar2=None,
                                op0=AluOpType.add)

    # ---- denominator: ones[128,128] matmul = partition-reduce + broadcast
    rc_sb = rcpool.tile([P, N], F32, tag="rc")
    rscr = rcpool.tile([P, FH], F32, tag="rscr")
    for ih in range(IH):
        cs_ps = p_misc.tile([P, FH], F32, tag="m")
        nc.tensor.matmul(cs_ps[:], aps["ones_sq"][:], acc_a[:, sl[ih]],
                         start=True, stop=False)
        nc.tensor.matmul(cs_ps[:], aps["ones_sq"][:], acc_b[:, sl[ih]],
                         start=False, stop=True)
        nc.vector.reciprocal_approx_accurate(out=rc_sb[:, sl[ih]],
                                             in_=cs_ps[:], scratch=rscr[:])

    # ---- copy (unnormalized) U to SBUF; normalization is deferred past Wo
    u_sb = upool.tile([P, CT, N], MM_DT, tag="u_sb")
    for ci in range(CT):
        for ih in range(IH):
            nc.scalar.activation(out=u_sb[:, ci, sl[ih]],
                                 in_=u_ps[ci * IH + ih][:], func=AF.Copy)

    # ---- output projection; then y = (Wo U') + (x + bo) ----
    y_sb = ypool.tile([P, CT, N], F32, tag="y")
    for ot in range(CT):
        for ih in range(IH):
            o_ps = p_misc.tile([P, FH], F32, tag="m")
            for ci in range(CT):
                nc.tensor.matmul(
                    o_ps[:],
                    aps["wo"][:, ci, ot * P:(ot + 1) * P],
                    u_sb[:, ci, sl[ih]],
                    start=(ci == 0), stop=(ci == CT - 1))
            nc.vector.tensor_mul(y_sb[:, ot, sl[ih]], o_ps[:],
                                 rc_sb[:, sl[ih]])
            nc.gpsimd.tensor_add(y_sb[:, ot, sl[ih]], y_sb[:, ot, sl[ih]],
                                 xb_sb[:, ot, sl[ih]])
            dma_eng = nc.sync if (ot + ih) % 2 == 0 else nc.scalar
            dma_eng.dma_start(out=aps["y"][b][:, ot, sl[ih]],
                              in_=y_sb[:, ot, sl[ih]])


def _build():
    nc = bacc.Bacc("TRN2", target_bir_lowering=False, debug=False,
                   enable_asserts=False, num_devices=N_CORES)

    x_d = nc.dram_tensor("x", [BPC, C, N], F32, kind="ExternalInput")
    y_d = nc.dram_tensor("y", [BPC, C, N], F32, kind="ExternalOutput")
    wall_d = nc.dram_tensor("wall", [3, C, C], MM_DT, kind="ExternalInput")
    cpack_d = nc.dram_tensor("cpack", [P, 16], F32, kind="ExternalInput")

    with tile.TileContext(nc) as tc:
        with (
            tc.tile_pool(name="consts", bufs=1) as consts,
            tc.tile_pool(name="xpool", bufs=2) as xpool,
            tc.tile_pool(name="npool", bufs=2) as npool,
            tc.tile_pool(name="qkpool", bufs=2) as qkpool,
            tc.tile_pool(name="vtpool", bufs=2) as vtpool,
            tc.tile_pool(name="etpool", bufs=3) as etpool,
            tc.tile_pool(name="accpool", bufs=2) as accpool,
            tc.tile_pool(name="rcpool", bufs=2) as rcpool,
            tc.tile_pool(name="upool", bufs=2) as upool,
            tc.tile_pool(name="ypool", bufs=2) as ypool,
            tc.tile_pool(name="xbpool", bufs=2) as xbpool,
            tc.tile_pool(name="small", bufs=4) as small,
            tc.tile_pool(name="p_st", bufs=2, space="PSUM") as p_st,
            tc.tile_pool(name="p_u", bufs=CT * IH, space="PSUM") as p_u,
            tc.tile_pool(name="p_misc", bufs=2, space="PSUM") as p_misc,
        ):
            aps = {}
            aps["x"] = x_d.ap().rearrange("b (t p) n -> b p t n", p=P)
            aps["y"] = y_d.ap().rearrange("b (t p) n -> b p t n", p=P)

            # x first (gates groupnorm) on the HWDGE sync queue
            # one packed const DMA: [P,16] f32 holds gnw|gnb|vq|bo|ind_fwd
            # (cols 0..11) and ind_bwd packed transposed in cols 12..13.
            # These are tiny and gate the groupnorm tail: issue them FIRST.
            cp = consts.tile([P, 16], F32, tag="cpack")
            nc.sync.dma_start(out=cp[:], in_=cpack_d.ap())
            aps["gnw"] = cp[:, 0:2]
            aps["gnb"] = cp[:, 2:4]
            aps["vq"] = cp[:, 4:6]
            aps["bo"] = cp[:, 8:10]
            aps["ind_fwd"] = cp[:, 10:12]
            ind_bwd = consts.tile([2, P], F32, tag="ind_bwd")
            nc.sync.dma_start(
                out=ind_bwd[:],
                in_=bass.AP(tensor=cpack_d, offset=12, ap=[[1, 2], [16, P]]))
            aps["ind_bwd"] = ind_bwd

            # weights first on the scalar ring (they gate the projections
            # and the PE's prefetched LDWEIGHTS), then the x tiles
            wall_t = consts.tile([P, 3, CT, C], MM_DT, tag="wall")
            nc.scalar.dma_start(
                out=wall_t[:],
                in_=wall_d.ap().rearrange("w (t p) o -> p w t o", p=P))
            for wi, dst in enumerate(("wm", "wv", "wo")):
                aps[dst] = wall_t[:, wi]

            aps["x_sb"] = []
            for b in range(BPC):
                tiles = []
                for t in range(CT):
                    x_tt = xpool.tile([P, N], F32, tag=f"x{t}",
                                      name=f"x_sb{b}_{t}")
                    dma_eng = nc.sync if t == 0 else nc.scalar
                    dma_eng.dma_start(out=x_tt[:], in_=aps["x"][b][:, t, :])
                    tiles.append(x_tt)
                aps["x_sb"].append(tiles)
            ones_sq = consts.tile([P, P], MM_DT, tag="ones_sq")
            nc.gpsimd.memset(ones_sq[:], 1.0)
            aps["ones_sq"] = ones_sq
            eps_t = consts.tile([2, 1], F32, tag="eps")
            nc.vector.memset(eps_t[:], EPS)
            aps["eps"] = eps_t
            warm = consts.tile([2, 4], F32, tag="actwarm")
            for wi, fn in enumerate((AF.Sqrt, AF.Identity, AF.Exp, AF.Copy)):
                nc.scalar.activation(out=warm[:, wi:wi + 1],
                                     in_=eps_t[:], func=fn)

            pools = (consts, xpool, npool, qkpool, vtpool, etpool, accpool,
                     rcpool, upool, ypool, xbpool, small, p_st, p_u, p_misc)
            for b in range(BPC):
                _build_gn(nc, tc, pools, aps, b)
            for b in range(BPC):
                _build_attn(nc, tc, pools, aps, b)

    nc.compile()
    return nc


_NC = None


def _get_nc():
    global _NC
    if _NC is None:
        _NC = _build()
    return _NC


def _make_in_maps(inputs):
    f32 = lambda a: np.ascontiguousarray(np.asarray(a, dtype=np.float32))
    mmdt = _np_mm_dt()
    wt = lambda a: np.asarray(a, dtype=np.float32).T.astype(mmdt)
    x = f32(inputs["x"]).reshape(B, C, N)
    wq64 = np.asarray(inputs["Wq"], np.float64)
    wk64 = np.asarray(inputs["Wk"], np.float64)
    # M^T = (Wk^T Wq)^T = Wq^T Wk, laid out [c', o] for the lhsT slot
    mT = (wq64.T @ wk64).astype(np.float32).astype(mmdt)
    wall = np.ascontiguousarray(np.stack(
        [np.ascontiguousarray(mT), wt(inputs["Wv"]), wt(inputs["Wo"])]))
    # softmax rows sum to 1 => the bv term reaches y as the constant
    # per-channel vector Wo @ bv; fold it into bo on the host.
    bo_eff = (np.asarray(inputs["bo"], np.float64)
              + np.asarray(inputs["Wo"], np.float64)
              @ np.asarray(inputs["bv"], np.float64)).astype(np.float32)
    pt = lambda a: f32(a).reshape(CT, P).T          # [256] -> [P, CT]
    cpack = np.zeros((P, 16), np.float32)
    cpack[:, 0:2] = pt(inputs["gn_w"])
    cpack[:, 2:4] = pt(inputs["gn_b"])
    vq = wk64.T @ np.asarray(inputs["bq"], np.float64)   # folds bq into P1
    cpack[:, 4:6] = pt(vq.astype(np.float32))
    cpack[:, 8:10] = pt(bo_eff)
    cpack[:GSIZE, 10] = 1.0 / GSIZE                 # ind_fwd
    cpack[GSIZE:, 11] = 1.0 / GSIZE
    cpack[:GSIZE, 12] = 1.0                         # ind_bwd (transposed)
    cpack[GSIZE:, 13] = 1.0
    shared = {"wall": wall, "cpack": cpack}

    in_maps = []
    for m in range(N_CORES):
        im = dict(shared)
        im["x"] = np.ascontiguousarray(x[m * BPC:(m + 1) * BPC])
        in_maps.append(im)
    return in_maps


def _gather(results):
    y = np.concatenate([r["y"] for r in results], axis=0)
    return np.ascontiguousarray(y.reshape(B, C, H, W).astype(np.float32))


def kernel(**inputs):
    nc = _get_nc()
    res = bass_utils.run_bass_kernel_spmd(nc, _make_in_maps(inputs),
                                          core_ids=list(range(N_CORES)))
    return _gather(res.results)


def _ensure_ntff_hook():
    """The agent image lacks antenv.axon_hooks; synthesize it and install the
    ctypes-based NTFF hook from trn_agent_boot so trace=True works locally."""
    import sys
    import types
    try:
        from antenv.axon_hooks import get_axon_ntff_profile_hook  # noqa: F401
        return
    except ImportError:
        pass
    hook = None
    try:
        from trn_agent_boot.trn_boot import _ntff_profile_via_ctypes
        hook = _ntff_profile_via_ctypes("/opt/axon/libaxon_pjrt.so")
    except Exception:
        hook = None
    mod = types.ModuleType("antenv.axon_hooks")
    mod.get_axon_ntff_profile_hook = lambda: hook
    mod.set_axon_ntff_profile_hook = lambda h: None
    sys.modules["antenv.axon_hooks"] = mod
    # keep artifacts local: no bucket in this sandbox
    bass_utils.upload_artifacts = lambda d: d


def kernel_traced(**inputs):
    """Returns (output, exec_time_ns, trace_path) using NTFF profiling."""
    _ensure_ntff_hook()
    nc = _get_nc()
    res = bass_utils.run_bass_kernel_spmd(nc, _make_in_maps(inputs),
                                          core_ids=list(range(N_CORES)),
                                          trace=True)
    trace_path = None
    if res.instructions_and_trace is not None:
        trace_path = res.instructions_and_trace[1]
    return _gather(res.results), res.exec_time_ns, trace_path


# revision 21
# speedup vs baseline: 1.0362x; 1.0128x over previous
"""Trainium2 Bass kernel for the GroupNorm + single-head spatial attention block.

Reference computation (per batch b):
    n  = GroupNorm(x, groups=4) * gn_w + gn_b          x: [C=256, N=1024]
    Q  = Wq @ n + bq ; K = Wk @ n + bk ; V = Wv @ n + bv
    S  = Q^T K / sqrt(C)                                [N, N]
    A  = softmax(S, axis=-1)
    U  = V @ A^T                                        [C, N]
    y  = x + Wo @ U + bo

Strategy (data-parallel over batch, 2 batches per NeuronCore, 8 cores):
  - matmul operands in bf16 (1 cycle/row on the PE, fp32 PSUM accumulation);
    everything else (stats, softmax denominator, residual) in fp32.
  - S is computed TRANSPOSED (S^T = K_tile^T @ Q, j on partitions) so the
    exp result E^T = exp(S^T/16) feeds U = V @ E^T directly (contraction
    over j on partitions).  No [N,N] transpose anywhere.
  - softmax skips the max-subtraction (|S|/16 is O(1), exp is safe).  The
    denominator (sum over j = partitions) is accumulated on DVE across
    j-tiles, then ONE ones[128,128] matmul both reduces over partitions and
    broadcasts to all 128 partitions.  The reciprocal is applied AFTER the
    Wo projection (per-output-column scaling commutes through the V
    contraction and Wo), so the PE never waits on it.
  - GroupNorm stats: bn_stats/bn_aggr per partition, then a [128,2]
    indicator matmul reduces (mean, E[x^2]) over each group's 64 partitions
    and a [2,128] indicator matmul broadcasts (mean, rstd) back.
  - engine balance: PE ~44us of matmul; ACT: exp, V^T copies, x+bo;
    DVE: bn stats, Q/K bias copies, colsum accumulation, reciprocal,
    U*recip; GpSimd: groupnorm apply, final residual add.
  - DMA: x on the sync (HWDGE) queue first; weights in parallel on the
    gpsimd (SWDGE) queue so compute starts ~3us in.
"""

import os
import numpy as np

import concourse.bass as bass
import concourse.bacc as bacc
import concourse.tile as tile
import concourse.bass_utils as bass_utils
from concourse import mybir
from concourse.alu_op_type import AluOpType

P = 128
B, C, H, W = 16, 256, 32, 32
N = H * W                 # 1024
N_CORES = 8
BPC = B // N_CORES        # batches per core
CT = C // P               # 2 c-tiles
JT = N // P               # 8 j-tiles
FH = 512                  # free-dim half (one PSUM bank of fp32)
IH = N // FH              # 2 i-halves
GROUPS = 4
GSIZE = C // GROUPS       # 64 channels per group
EPS = 1e-5
SCALE = 1.0 / float(np.sqrt(C))

F32 = mybir.dt.float32
BF16 = mybir.dt.bfloat16

AF = mybir.ActivationFunctionType

# matmul operand dtype: bf16 (fast) or f32 (exact, 4 cycles/row)
MODE = os.environ.get("ATTN_DT", "bf16")
MM_DT = {"bf16": BF16, "f32": F32}[MODE]


def _np_mm_dt():
    import ml_dtypes
    return {"bf16": ml_dtypes.bfloat16, "f32": np.float32}[MODE]


def _build_gn(nc, tc, pools, aps, b):
    """GroupNorm + xb for batch b (hoisted so both batches' DVE work runs
    up front, keeping the later batch's matmuls ready when the PE frees)."""
    (consts, xpool, npool, qkpool, vtpool, etpool, accpool, rcpool, upool,
     ypool, xbpool, small, p_st, p_u, p_misc) = pools

    sl = [slice(ih * FH, (ih + 1) * FH) for ih in range(IH)]
    x_t = aps["x_sb"][b]          # list of CT tiles [P, N]

    # ---- GroupNorm (both c-tiles' chains batched into strided ops) ----
    n_sb = npool.tile([P, CT, N], MM_DT, tag="n")
    mv_all = small.tile([P, CT, 2], F32, tag="mv")
    for t in range(CT):
        resh = x_t[t][:].rearrange("p (s f) -> p s f", f=FH)
        stats6 = small.tile([P, IH, 6], F32, tag=f"stats6_{t}")
        for s in range(IH):
            nc.vector.bn_stats(out=stats6[:, s, :], in_=resh[:, s, :])
        nc.vector.bn_aggr(out=mv_all[:, t, :], in_=stats6[:])
    # pq = (mean, E[x^2]) per partition, both tiles side by side
    pq = small.tile([P, CT, 2], F32, tag="pq")
    nc.vector.tensor_copy(pq[:, :, 0], mv_all[:, :, 0])
    nc.vector.tensor_mul(pq[:, :, 1], mv_all[:, :, 0], mv_all[:, :, 0])
    nc.vector.tensor_add(pq[:, :, 1], pq[:, :, 1], mv_all[:, :, 1])
    # group-reduce over partitions: [2, (t,k)] = (mean_g, E[x^2]_g)
    stats_ps = p_misc.tile([2, CT, 2], F32, tag="m")
    nc.tensor.matmul(stats_ps[:], aps["ind_fwd"][:], pq[:],
                     start=True, stop=True)
    s_sb = small.tile([2, CT, 2], F32, tag="s2")
    nc.vector.tensor_copy(s_sb[:], stats_ps[:])
    msq = small.tile([2, CT], F32, tag="msq")
    nc.vector.tensor_mul(msq[:], s_sb[:, :, 0], s_sb[:, :, 0])
    nc.vector.tensor_sub(msq[:], s_sb[:, :, 1], msq[:])         # var
    nc.scalar.activation(out=msq[:], in_=msq[:], func=AF.Sqrt,
                         bias=aps["eps"][:])                    # sqrt(var+eps)
    nc.vector.reciprocal(out=s_sb[:, :, 1], in_=msq[:])         # rstd
    # broadcast (mean, rstd) to the 128 partitions
    bc_ps = p_misc.tile([P, CT, 2], F32, tag="m")
    nc.tensor.matmul(bc_ps[:], aps["ind_bwd"][:], s_sb[:],
                     start=True, stop=True)
    # fold gamma/beta: n = x * (rstd*w) + (b - mean*rstd*w)
    sc = small.tile([P, CT, 2], F32, tag="sc")
    nc.vector.tensor_mul(sc[:, :, 0], bc_ps[:, :, 1], aps["gnw"])
    nc.vector.tensor_mul(sc[:, :, 1], bc_ps[:, :, 0], sc[:, :, 0])
    nc.vector.tensor_sub(sc[:, :, 1], aps["gnb"], sc[:, :, 1])
    for t in range(CT):
        nc.scalar.activation(out=n_sb[:, t, :], in_=x_t[t][:],
                             func=AF.Identity,
                             scale=sc[:, t, 0:1], bias=sc[:, t, 1:2])
    aps.setdefault("n_sb", {})[b] = n_sb


def _build_attn(nc, tc, pools, aps, b):
    """Projections + attention + output for batch b."""
    (consts, xpool, npool, qkpool, vtpool, etpool, accpool, rcpool, upool,
     ypool, xbpool, small, p_st, p_u, p_misc) = pools

    sl = [slice(ih * FH, (ih + 1) * FH) for ih in range(IH)]
    x_t = aps["x_sb"][b]
    n_sb = aps["n_sb"][b]

    # ---- merged QK projection: S^T = n^T (Wk^T Wq) n, so compute
    # P1 = M n + v with M = Wk^T Wq and v = Wk^T bq (both host-side).
    # The bk/bq cross terms are constant per softmax row and cancel.
    p1_sb = qkpool.tile([P, CT, N], MM_DT, tag="p1")
    for ot in range(CT):
        for ih in range(IH):
            pr_ps = p_misc.tile([P, FH], F32, tag="m")
            for kt in range(CT):
                nc.tensor.matmul(
                    pr_ps[:],
                    aps["wm"][:, kt, ot * P:(ot + 1) * P],
                    n_sb[:, kt, sl[ih]],
                    start=(kt == 0), stop=(kt == CT - 1))
            nc.vector.tensor_scalar(
                out=p1_sb[:, ot, sl[ih]], in0=pr_ps[:],
                scalar1=aps["vq"][:, ot:ot + 1], scalar2=None,
                op0=AluOpType.add)

    # ---- V^T: [N, C] (j on partitions), computed directly as n^T @ Wv^T ----
    # (bias bv is folded into the residual on the host: softmax rows sum to 1,
    #  so V*A^T with V = V0 + bv x 1 contributes exactly Wo@bv per channel.)
    vt_sb = vtpool.tile([P, JT, C], MM_DT, tag="vt")
    for jt in range(JT):
        vt_ps = p_misc.tile([P, C], F32, tag="m")
        for kt in range(CT):
            nc.tensor.matmul(vt_ps[:],
                             n_sb[:, kt, jt * P:(jt + 1) * P],
                             aps["wv"][:, kt, :],
                             start=(kt == 0), stop=(kt == CT - 1))
        nc.vector.tensor_copy(vt_sb[:, jt, :], vt_ps[:])

    # ---- attention: S^T -> exp -> (colsum, U-accumulate) per j-tile ----
    u_ps = [p_u.tile([P, FH], F32, tag="u", name=f"u_ps{b}_{i}")
            for i in range(CT * IH)]
    acc_a = accpool.tile([P, N], MM_DT, tag="acc_a")
    acc_b = accpool.tile([P, N], MM_DT, tag="acc_b")
    for jt in range(JT):
        et = etpool.tile([P, N], MM_DT, tag="et")
        for ih in range(IH):
            st_ps = p_st.tile([P, FH], F32, tag="st")
            for kt in range(CT):
                nc.tensor.matmul(
                    st_ps[:],
                    n_sb[:, kt, jt * P:(jt + 1) * P],
                    p1_sb[:, kt, sl[ih]],
                    start=(kt == 0), stop=(kt == CT - 1))
            nc.scalar.activation(out=et[:, sl[ih]], in_=st_ps[:],
                                 func=AF.Exp, scale=SCALE)
        if jt == 0:
            nc.vector.tensor_copy(acc_a[:], et[:])
        elif jt == 1:
            nc.vector.tensor_copy(acc_b[:], et[:])
        elif jt % 2 == 0:
            nc.vector.tensor_add(acc_a[:], acc_a[:], et[:])
        else:
            nc.vector.tensor_add(acc_b[:], acc_b[:], et[:])
        for ci in range(CT):
            for ih in range(IH):
                nc.tensor.matmul(
                    u_ps[ci * IH + ih][:],
                    vt_sb[:, jt, ci * P:(ci + 1) * P],
                    et[:, sl[ih]],
                    start=(jt == 0), stop=(jt == JT - 1))

    # ---- xb = x + bo (per-partition bias), used by the final residual ----
    xb_sb = xbpool.tile([P, CT, N], F32, tag="xb")
    for ot in range(CT):
        nc.vector.tensor_scalar(out=xb_sb[:, ot, :], in0=x_t[ot][:],
                                scalar1=aps["bo"][:, ot:ot + 1], scalar2=None,
                                op0=AluOpType.add)

    # ---- denominator: ones[128,128] matmul = partition-reduce + broadcast
    rc_sb = rcpool.tile([P, N], F32, tag="rc")
    rscr = rcpool.tile([P, FH], F32, tag="rscr")
    for ih in range(IH):
        cs_ps = p_misc.tile([P, FH], F32, tag="m")
        nc.tensor.matmul(cs_ps[:], aps["ones_sq"][:], acc_a[:, sl[ih]],
                         start=True, stop=False)
        nc.tensor.matmul(cs_ps[:], aps["ones_sq"][:], acc_b[:, sl[ih]],
                         start=False, stop=True)
        nc.vector.reciprocal_approx_accurate(out=rc_sb[:, sl[ih]],
                                             in_=cs_ps[:], scratch=rscr[:])

    # ---- copy (unnormalized) U to SBUF; normalization is deferred past Wo
    u_sb = upool.tile([P, CT, N], MM_DT, tag="u_sb")
    for ci in range(CT):
        for ih in range(IH):
            nc.scalar.activation(out=u_sb[:, ci, sl[ih]],
                                 in_=u_ps[ci * IH + ih][:], func=AF.Copy)

    # ---- output projection; then y = (Wo U') + (x + bo) ----
    y_sb = ypool.tile([P, CT, N], F32, tag="y")
    for ot in range(CT):
        for ih in range(IH):
            o_ps = p_misc.tile([P, FH], F32, tag="m")
            for ci in range(CT):
                nc.tensor.matmul(
                    o_ps[:],
                    aps["wo"][:, ci, ot * P:(ot + 1) * P],
                    u_sb[:, ci, sl[ih]],
                    start=(ci == 0), stop=(ci == CT - 1))
            nc.vector.tensor_mul(y_sb[:, ot, sl[ih]], o_ps[:],
                                 rc_sb[:, sl[ih]])
            nc.gpsimd.tensor_add(y_sb[:, ot, sl[ih]], y_sb[:, ot, sl[ih]],
                                 xb_sb[:, ot, sl[ih]])
            dma_eng = nc.sync if (ot + ih) % 2 == 0 else nc.scalar
            dma_eng.dma_start(out=aps["y"][b][:, ot, sl[ih]],
                              in_=y_sb[:, ot, sl[ih]])


def _build():
    nc = bacc.Bacc("TRN2", target_bir_lowering=False, debug=False,
                   enable_asserts=False, num_devices=N_CORES)

    x_d = nc.dram_tensor("x", [BPC, C, N], F32, kind="ExternalInput")
    y_d = nc.dram_tensor("y", [BPC, C, N], F32, kind="ExternalOutput")
    wall_d = nc.dram_tensor("wall", [3, C, C], MM_DT, kind="ExternalInput")
    cpack_d = nc.dram_tensor("cpack", [P, 16], F32, kind="ExternalInput")

    with tile.TileContext(nc) as tc:
        with (
            tc.tile_pool(name="consts", bufs=1) as consts,
            tc.tile_pool(name="xpool", bufs=2) as xpool,
            tc.tile_pool(name="npool", bufs=2) as npool,
            tc.tile_pool(name="qkpool", bufs=2) as qkpool,
            tc.tile_pool(name="vtpool", bufs=2) as vtpool,
            tc.tile_pool(name="etpool", bufs=3) as etpool,
            tc.tile_pool(name="accpool", bufs=2) as accpool,
            tc.tile_pool(name="rcpool", bufs=2) as rcpool,
            tc.tile_pool(name="upool", bufs=2) as upool,
            tc.tile_pool(name="ypool", bufs=2) as ypool,
            tc.tile_pool(name="xbpool", bufs=2) as xbpool,
            tc.tile_pool(name="small", bufs=4) as small,
            tc.tile_pool(name="p_st", bufs=2, space="PSUM") as p_st,
            tc.tile_pool(name="p_u", bufs=CT * IH, space="PSUM") as p_u,
            tc.tile_pool(name="p_misc", bufs=2, space="PSUM") as p_misc,
        ):
            aps = {}
            aps["x"] = x_d.ap().rearrange("b (t p) n -> b p t n", p=P)
            aps["y"] = y_d.ap().rearrange("b (t p) n -> b p t n", p=P)

            # x first (gates groupnorm) on the HWDGE sync queue
            # Ring order matters: ~2 outstanding DMAs per HWDGE ring and
            # ~2us completion latency each.  b0's x halves lead both rings
            # (they gate groupnorm); consts/weights ride second.
            aps["x_sb"] = [[None] * CT for _ in range(BPC)]
            for b in range(BPC):
                for t in range(CT):
                    aps["x_sb"][b][t] = xpool.tile(
                        [P, N], F32, tag=f"x{t}", name=f"x_sb{b}_{t}")

            nc.sync.dma_start(out=aps["x_sb"][0][0][:],
                              in_=aps["x"][0][:, 0, :])
            nc.scalar.dma_start(out=aps["x_sb"][0][1][:],
                                in_=aps["x"][0][:, 1, :])

            # packed consts: [P,16] f32 holds gnw|gnb|vq|bo|ind_fwd (cols
            # 0..11) and ind_bwd packed transposed in cols 12..13.
            cp = consts.tile([P, 16], F32, tag="cpack")
            nc.sync.dma_start(out=cp[:], in_=cpack_d.ap())
            aps["gnw"] = cp[:, 0:2]
            aps["gnb"] = cp[:, 2:4]
            aps["vq"] = cp[:, 4:6]
            aps["bo"] = cp[:, 8:10]
            aps["ind_fwd"] = cp[:, 10:12]

            wall_t = consts.tile([P, 3, CT, C], MM_DT, tag="wall")
            nc.scalar.dma_start(
                out=wall_t[:],
                in_=wall_d.ap().rearrange("w (t p) o -> p w t o", p=P))
            for wi, dst in enumerate(("wm", "wv", "wo")):
                aps[dst] = wall_t[:, wi]

            ind_bwd = consts.tile([2, P], F32, tag="ind_bwd")
            nc.sync.dma_start(
                out=ind_bwd[:],
                in_=bass.AP(tensor=cpack_d, offset=12, ap=[[1, 2], [16, P]]))
            aps["ind_bwd"] = ind_bwd

            nc.sync.dma_start(out=aps["x_sb"][1][0][:],
                              in_=aps["x"][1][:, 0, :])
            nc.scalar.dma_start(out=aps["x_sb"][1][1][:],
                                in_=aps["x"][1][:, 1, :])
            ones_sq = consts.tile([P, P], MM_DT, tag="ones_sq")
            nc.gpsimd.memset(ones_sq[:], 1.0)
            aps["ones_sq"] = ones_sq
            eps_t = consts.tile([2, 1], F32, tag="eps")
            nc.vector.memset(eps_t[:], EPS)
            aps["eps"] = eps_t
            warm = consts.tile([2, 4], F32, tag="actwarm")
            for wi, fn in enumerate((AF.Sqrt, AF.Identity, AF.Exp, AF.Copy)):
                nc.scalar.activation(out=warm[:, wi:wi + 1],
                                     in_=eps_t[:], func=fn)

            pools = (consts, xpool, npool, qkpool, vtpool, etpool, accpool,
                     rcpool, upool, ypool, xbpool, small, p_st, p_u, p_misc)
            for b in range(BPC):
                _build_gn(nc, tc, pools, aps, b)
            for b in range(BPC):
                _build_attn(nc, tc, pools, aps, b)

    nc.compile()
    return nc


_NC = None


def _get_nc():
    global _NC
    if _NC is None:
        _NC = _build()
    return _NC


def _make_in_maps(inputs):
    f32 = lambda a: np.ascontiguousarray(np.asarray(a, dtype=np.float32))
    mmdt = _np_mm_dt()
    wt = lambda a: np.asarray(a, dtype=np.float32).T.astype(mmdt)
    x = f32(inputs["x"]).reshape(B, C, N)
    wq64 = np.asarray(inputs["Wq"], np.float64)
    wk64 = np.asarray(inputs["Wk"], np.float64)
    # M^T = (Wk^T Wq)^T = Wq^T Wk, laid out [c', o] for the lhsT slot
    mT = (wq64.T @ wk64).astype(np.float32).astype(mmdt)
    wall = np.ascontiguousarray(np.stack(
        [np.ascontiguousarray(mT), wt(inputs["Wv"]), wt(inputs["Wo"])]))
    # softmax rows sum to 1 => the bv term reaches y as the constant
    # per-channel vector Wo @ bv; fold it into bo on the host.
    bo_eff = (np.asarray(inputs["bo"], np.float64)
              + np.asarray(inputs["Wo"], np.float64)
              @ np.asarray(inputs["bv"], np.float64)).astype(np.float32)
    pt = lambda a: f32(a).reshape(CT, P).T          # [256] -> [P, CT]
    cpack = np.zeros((P, 16), np.float32)
    cpack[:, 0:2] = pt(inputs["gn_w"])
    cpack[:, 2:4] = pt(inputs["gn_b"])
    vq = wk64.T @ np.asarray(inputs["bq"], np.float64)   # folds bq into P1
    cpack[:, 4:6] = pt(vq.astype(np.float32))
    cpack[:, 8:10] = pt(bo_eff)
    cpack[:GSIZE, 10] = 1.0 / GSIZE                 # ind_fwd
    cpack[GSIZE:, 11] = 1.0 / GSIZE
    cpack[:GSIZE, 12] = 1.0                         # ind_bwd (transposed)
    cpack[GSIZE:, 13] = 1.0
    shared = {"wall": wall, "cpack": cpack}

    in_maps = []
    for m in range(N_CORES):
        im = dict(shared)
        im["x"] = np.ascontiguousarray(x[m * BPC:(m + 1) * BPC])
        in_maps.append(im)
    return in_maps


def _gather(results):
    y = np.concatenate([r["y"] for r in results], axis=0)
    return np.ascontiguousarray(y.reshape(B, C, H, W).astype(np.float32))


def kernel(**inputs):
    nc = _get_nc()
    res = bass_utils.run_bass_kernel_spmd(nc, _make_in_maps(inputs),
                                          core_ids=list(range(N_CORES)))
    return _gather(res.results)


def _ensure_ntff_hook():
    """The agent image lacks antenv.axon_hooks; synthesize it and install the
    ctypes-based NTFF hook from trn_agent_boot so trace=True works locally."""
    import sys
    import types
    try:
        from antenv.axon_hooks import get_axon_ntff_profile_hook  # noqa: F401
        return
    except ImportError:
        pass
    hook = None
    try:
        from trn_agent_boot.trn_boot import _ntff_profile_via_ctypes
        hook = _ntff_profile_via_ctypes("/opt/axon/libaxon_pjrt.so")
    except Exception:
        hook = None
    mod = types.ModuleType("antenv.axon_hooks")
    mod.get_axon_ntff_profile_hook = lambda: hook
    mod.set_axon_ntff_profile_hook = lambda h: None
    sys.modules["antenv.axon_hooks"] = mod
    # keep artifacts local: no bucket in this sandbox
    bass_utils.upload_artifacts = lambda d: d


def kernel_traced(**inputs):
    """Returns (output, exec_time_ns, trace_path) using NTFF profiling."""
    _ensure_ntff_hook()
    nc = _get_nc()
    res = bass_utils.run_bass_kernel_spmd(nc, _make_in_maps(inputs),
                                          core_ids=list(range(N_CORES)),
                                          trace=True)
    trace_path = None
    if res.instructions_and_trace is not None:
        trace_path = res.instructions_and_trace[1]
    return _gather(res.results), res.exec_time_ns, trace_path
